# revision 33
# baseline (speedup 1.0000x reference)
"""Trainium2 Bass kernel for nn_MixtureLayer (MoE routing, 8 experts, top-2,
grouped capacity routing + shared expert).

Strategy: data-parallel over the 128 token-groups -> 16 groups per core.
Each core runs the router, dispatch, all 8 experts' FFNs on its own groups,
the shared expert, and the combine.  No collectives needed.

Numerics: router (logits/softmax/top-k/cumsum) entirely in fp32 so expert
selection matches the jax reference; the heavy FFN matmuls run in bf16 with
fp32 PSUM accumulation.  All FFN weights are pre-cast to bf16 on the host so
the device streams half the bytes and spends no vector cycles casting.

Schedule: the 4 shared-expert FFN units are interleaved into the router loop
(unit q emitted after groups 4q..4q+3) so the PE stays busy while the vector
engine runs each group's serial softmax/top-k/cumsum chain.
"""

import sys
import types

import numpy as np
import ml_dtypes

try:  # concourse is normally on sys.path via the container's site setup
    import concourse.bass as bass  # noqa: F401
except ImportError:  # pragma: no cover
    sys.path.insert(0, "/opt/trn_rl_repo")

import concourse.bass as bass
import concourse.tile as tile
from concourse import bacc, mybir
from concourse.bass_utils import run_bass_kernel_spmd

F32 = mybir.dt.float32
BF16 = mybir.dt.bfloat16
AF = mybir.ActivationFunctionType
ALU = mybir.AluOpType
GELU = AF.Gelu_apprx_tanh  # jax.nn.gelu(approximate=True)

# ---- problem constants (hardcoded from the spec) ----
NCORES = 8
D, H, E = 1024, 4096, 8
B, S = 8, 2048
GRP = 128                 # tokens per routing group
NG_TOT = 128              # total groups
NG = NG_TOT // NCORES     # groups per core = 16
TOK = NG * GRP            # tokens per core = 2048
CAP = 32                  # capacity slots per (group, expert); slot 31 unused
DC = D // 128             # 8 chunks of d
HC = H // 128             # 32 chunks of h
SLOTS = NG * CAP          # 512 slots per expert per core

_CACHE = {}


def _ensure_ntff_hook():
    """Register the axon NTFF profiling hook if the image's antenv stub lacks
    it (needed only when tracing; harmless otherwise)."""
    try:
        import antenv
    except ImportError:
        return
    if "antenv.axon_hooks" in sys.modules:
        return
    m = types.ModuleType("antenv.axon_hooks")
    m._hook = None

    def _set(h, _m=m):
        _m._hook = h

    def _get(_m=m):
        return _m._hook

    m.set_axon_ntff_profile_hook = _set
    m.get_axon_ntff_profile_hook = _get
    sys.modules["antenv.axon_hooks"] = m
    antenv.axon_hooks = m
    try:
        from trn_agent_boot.trn_boot import _ntff_profile_via_ctypes

        hook = _ntff_profile_via_ctypes("/opt/axon/libaxon_pjrt.so")
        if hook is not None:
            _set(hook)
    except Exception:
        pass


def _emit_ffn_unit(nc, pools, rhs_fn, out_ap_fn, keys_ap, vals_ap,
                   pre_ffn2_cb=None):
    """One FFN 'unit': 512 input columns (slots/tokens) through d->h gelu h->d.

    rhs_fn(dc) -> AP [128, 512] of the input in transposed layout (d on
    partitions).  Weights stream directly from host-precast bf16 HBM:
    keys_ap [1024, 4096] bf16, vals_ap [4096, 1024] bf16.
    """
    hid = [pools["hid"].tile([128, 512], BF16, tag=f"hid{hc}", name=f"hid{hc}") for hc in range(HC)]

    # FFN2's values stream: pairs of h-chunks per DMA, all on the sync
    # queue.  The first two pair-triggers are emitted BEFORE FFN1 so they
    # fire while the previous unit's FFN2 drains and land at FFN1 start;
    # the rest are emitted inline in FFN2 (they fire once the keys stream
    # drains, staying ahead of consumption).
    vals_3d = vals_ap.rearrange("(hp i q) d -> hp q i d", i=2, q=128)
    vb2 = [None] * 16

    def fetch_vb2(hp):
        vb2[hp] = pools["vb"].tile([128, 2, 1024], BF16, tag="vb", name="vb")
        nc.sync.dma_start(vb2[hp][:], vals_3d[hp])

    fetch_vb2(0)
    fetch_vb2(1)

    # FFN1: hid[hc][128, 512] = gelu(sum_dc keys[dc,hc].T @ rhs[dc])
    for hcb in range(8):  # blocks of 4 h-chunks
        eps = [pools["ps"].tile([128, 512], F32, tag="ps", name="ps") for _ in range(4)]
        for dc in range(DC):
            kb = pools["kb"].tile([128, 512], BF16, tag="kb", name="kb")
            nc.sync.dma_start(kb[:], keys_ap[dc * 128:(dc + 1) * 128,
                                             hcb * 512:(hcb + 1) * 512])
            rhs = rhs_fn(dc)
            for hh in range(4):
                nc.tensor.matmul(eps[hh][:], kb[:, hh * 128:(hh + 1) * 128], rhs,
                                 start=(dc == 0), stop=(dc == DC - 1))
        for hh in range(4):
            nc.scalar.activation(hid[hcb * 4 + hh][:], eps[hh][:], GELU)
    if pre_ffn2_cb is not None:
        pre_ffn2_cb()
    # FFN2: out[sc*128.., 1024] = sum_hc hid[hc][:,sc].T @ values[hc]
    pss = [[pools["ps"].tile([128, 512], F32, tag="ps", name="ps") for _ in range(2)]
           for _ in range(4)]
    for hc in range(HC):
        if hc % 2 == 0 and hc // 2 + 2 < 16:
            fetch_vb2(hc // 2 + 2)
        vb = vb2[hc // 2][:, hc % 2, :]
        for sc in range(4):
            lhsT = hid[hc][:, sc * 128:(sc + 1) * 128]
            nc.tensor.matmul(pss[sc][0][:], lhsT, vb[:, 0:512],
                             start=(hc == 0), stop=(hc == HC - 1))
            nc.tensor.matmul(pss[sc][1][:], lhsT, vb[:, 512:1024],
                             start=(hc == 0), stop=(hc == HC - 1))
    for sc in range(4):
        eo = pools["eo"].tile([128, 1024], BF16, tag="eo", name="eo")
        nc.scalar.copy(eo[:, 0:512], pss[sc][0][:])
        nc.scalar.copy(eo[:, 512:1024], pss[sc][1][:])
        nc.gpsimd.dma_start(out_ap_fn(sc), eo[:])


def _build_program():
    nc = bacc.Bacc("TRN2", target_bir_lowering=False, debug=False,
                   num_devices=NCORES)

    x_d = nc.dram_tensor("x_s", [TOK, D], F32, kind="ExternalInput").ap()
    gw_d = nc.dram_tensor("gw", [D, E], F32, kind="ExternalInput").ap()
    gb_d = nc.dram_tensor("gb", [1, E], F32, kind="ExternalInput").ap()
    k_d = nc.dram_tensor("k16", [E, D, H], BF16, kind="ExternalInput").ap()
    v_d = nc.dram_tensor("v16", [E, H, D], BF16, kind="ExternalInput").ap()
    sk_d = nc.dram_tensor("shk16", [D, H], BF16, kind="ExternalInput").ap()
    sv_d = nc.dram_tensor("shv16", [H, D], BF16, kind="ExternalInput").ap()
    out_d = nc.dram_tensor("out", [TOK, D], BF16, kind="ExternalOutput").ap()

    from contextlib import ExitStack
    with tile.TileContext(nc) as tc, ExitStack() as es_glob:
        # pool releases must be LIFO; phases close explicitly in stack order
        es_xtb, es_dT = ExitStack(), ExitStack()
        es_ffn, es_rt, es_cb = ExitStack(), ExitStack(), ExitStack()
        def mk(es, name, bufs, space="SBUF"):
            return es.enter_context(tc.tile_pool(name=name, bufs=bufs,
                                                 space=space))

        # global pools (live for whole kernel)
        ps = mk(es_glob, "ps", 8, "PSUM")
        const = mk(es_glob, "const", 1)
        dram = mk(es_glob, "dram", 1, "DRAM")
        p_ct = mk(es_glob, "p_ct", 1)
        pools = {"ps": ps}

        # ---------- persistent tensors ----------
        p_xtb = mk(es_xtb, "p_xtb", 1)
        p_dT = mk(es_dT, "p_dT", 1)
        xTb = [p_xtb.tile([128, TOK], BF16, tag=f"xtb{dc}", name=f"xtb{dc}")
               for dc in range(DC)]
        combT = [p_ct.tile([128, NG * 128], BF16, tag=f"ct{ch}",
                           name=f"ct{ch}") for ch in range(2)]
        dispT = [p_dT.tile([128, NG * E * CAP], BF16, tag=f"dT{dc}",
                           name=f"dT{dc}") for dc in range(DC)]

        # DRAM scratch, split per token-chunk so the combine phase can start
        # on early chunks while later experts are still writing late chunks
        eo_dram = [[dram.tile([512, D], BF16, tag=f"eo_dram{h}_{sc}",
                              name=f"eo_dram{h}_{sc}") for sc in range(4)]
                   for h in range(2)]
        sh_dram = [dram.tile([512, D], BF16, tag=f"sh_dram{q}",
                             name=f"sh_dram{q}") for q in range(4)]

        # ---------- FFN pools (created early: shared units interleave with
        # the router loop) ----------
        pools["kb"] = mk(es_ffn, "p_kb", 14)
        pools["vb"] = mk(es_ffn, "p_vb", 3)
        pools["hid"] = mk(es_ffn, "p_hid", 1)
        pools["eo"] = mk(es_ffn, "p_eo", 2)

        # ---------- router pools ----------
        p_xg = mk(es_rt, "p_xg", 2)
        p_xgb = mk(es_rt, "p_xgb", 5)
        p_dm = mk(es_rt, "p_dm", 3)
        p_xtf = mk(es_rt, "p_xtf", 2)
        p_sm = mk(es_rt, "p_sm", 8)
        p_sm8 = mk(es_rt, "p_sm8", 8)
        p_cmp = mk(es_rt, "p_cmp", 4)
        p_comb = mk(es_rt, "p_comb", 3)

        # ---------- prefetch x for the first groups before const setup;
        # group 0 split across both HWDGE queues so its first half (used by
        # the first transpose pack) lands sooner ----------
        xg_pre = []
        xg0 = p_xg.tile([128, D], F32, tag="xg", name="xg")
        nc.sync.dma_start(xg0[:, 0:512], x_d[0:128, 0:512])
        nc.scalar.dma_start(xg0[:, 512:1024], x_d[0:128, 512:1024])
        xg_pre.append(xg0)
        xg1 = p_xg.tile([128, D], F32, tag="xg", name="xg")
        nc.sync.dma_start(xg1[:], x_d[128:256, :])
        xg_pre.append(xg1)

        # ---------- constants ----------
        ones128 = const.tile([128, 128], F32, tag="ones128", name="ones128")
        nc.gpsimd.memset(ones128[:], 1.0)
        ident = const.tile([128, 128], F32, tag="ident", name="ident")
        nc.gpsimd.affine_select(ident[:], ones128[:], pattern=[[1, 128]],
                                base=0, channel_multiplier=-1,
                                compare_op=ALU.is_equal, fill=0.0)
        utri = const.tile([128, 128], F32, tag="utri", name="utri")
        nc.gpsimd.affine_select(utri[:], ones128[:], pattern=[[1, 128]],
                                base=0, channel_multiplier=-1,
                                compare_op=ALU.is_ge, fill=0.0)
        # iota over capacity slots: value c+1 at slot c (c<31), -1 at c=31
        iota_f = const.tile([128, E * CAP], F32, tag="iota_f", name="iota_f")
        nc.gpsimd.iota(iota_f[:], pattern=[[0, E], [1, CAP]], base=1,
                       channel_multiplier=0,
                       allow_small_or_imprecise_dtypes=True)
        iota_3d = iota_f[:].rearrange("p (e c) -> p e c", e=E)
        nc.vector.memset(iota_3d[:, :, CAP - 1:CAP], -1.0)
        gw_sb = const.tile([128, DC * E], F32, tag="gw", name="gw")
        for dc in range(DC):
            nc.scalar.dma_start(gw_sb[:, dc * E:(dc + 1) * E],
                                gw_d[dc * 128:(dc + 1) * 128, :])
        gb_sb = const.tile([1, E], F32, tag="gb", name="gb")
        nc.scalar.dma_start(gb_sb[:], gb_d[:])
        ones1 = const.tile([1, 128], F32, tag="ones1", name="ones1")
        nc.vector.memset(ones1[:], 1.0)

        # ---------- router + dispatch, interleaved with the shared expert.
        # Each group is emitted in three pieces so the in-order PE queue
        # never waits on the vector engine's serial softmax/top-k chain:
        #   head:   transposes + logits matmuls + the vector chain
        #   tail_a: cumsum matmuls + dispatch (vector results long ready)
        #   tail_b: combine-weight transpose (after the shared FFN unit)
        st = {}

        def emit_head(g):
            if g < len(xg_pre):
                xg = xg_pre[g]
            else:
                xg = p_xg.tile([128, D], F32, tag="xg", name="xg")
                nc.sync.dma_start(xg[:], x_d[g * 128:(g + 1) * 128, :])
            xgb = p_xgb.tile([128, D], BF16, tag="xgb", name="xgb")
            nc.scalar.copy(xgb[:], xg[:])

            # transpose x group: pack 4 [128,128] transposes per PSUM bank
            xtf = []
            for dc4 in range(2):
                tp = ps.tile([128, 512], F32, tag="ps", name="ps")
                for j in range(4):
                    dc = dc4 * 4 + j
                    nc.tensor.transpose(tp[:, j * 128:(j + 1) * 128],
                                        xg[:, dc * 128:(dc + 1) * 128],
                                        ident[:])
                t = p_xtf.tile([128, 512], F32, tag="xtf", name="xtf")
                nc.vector.tensor_copy(t[:], tp[:])
                xtf.append(t)
                for j in range(4):
                    dc = dc4 * 4 + j
                    nc.scalar.copy(xTb[dc][:, g * 128:(g + 1) * 128],
                                   tp[:, j * 128:(j + 1) * 128])
            lp = ps.tile([128, E], F32, tag="ps", name="ps")
            for dc in range(DC):
                nc.tensor.matmul(lp[:], xtf[dc // 4][:, (dc % 4) * 128:
                                                     (dc % 4 + 1) * 128],
                                 gw_sb[:, dc * E:(dc + 1) * E],
                                 start=(dc == 0), stop=False)
            nc.tensor.matmul(lp[:], ones1[:], gb_sb[:],
                             start=False, stop=True)
            # free the PSUM bank immediately; the chain reads from SBUF
            lg = p_sm8.tile([128, E], F32, tag="lg", name="lg")
            nc.vector.tensor_copy(lg[:], lp[:])
            negm = p_sm.tile([128, 1], F32, tag="negm", name="negm")
            nc.vector.tensor_reduce(negm[:], lg[:],
                                    axis=mybir.AxisListType.X,
                                    op=ALU.max, negate=True)
            ex = p_sm8.tile([128, E], F32, tag="ex", name="ex")
            den = p_sm.tile([128, 1], F32, tag="den", name="den")
            nc.scalar.activation(ex[:], lg[:], AF.Exp, bias=negm[:],
                                 scale=1.0, accum_out=den[:])
            rec = p_sm.tile([128, 1], F32, tag="rec", name="rec")
            nc.vector.reciprocal(rec[:], den[:])
            probs = p_sm8.tile([128, E], F32, tag="probs", name="probs")
            nc.vector.tensor_scalar_mul(probs[:], ex[:], rec[:])
            m1 = p_sm.tile([128, 1], F32, tag="m1", name="m1")
            nc.vector.reduce_max(m1[:], probs[:], axis=mybir.AxisListType.X)
            mask1 = p_sm8.tile([128, E], F32, tag="mask1", name="mask1")
            nc.vector.tensor_scalar(mask1[:], probs[:], m1[:], None,
                                    op0=ALU.is_ge)
            probs2 = p_sm8.tile([128, E], F32, tag="probs2", name="probs2")
            nc.vector.scalar_tensor_tensor(probs2[:], mask1[:], -1e30,
                                           probs[:], ALU.mult, ALU.add)
            m2 = p_sm.tile([128, 1], F32, tag="m2", name="m2")
            nc.vector.reduce_max(m2[:], probs2[:], axis=mybir.AxisListType.X)
            mask2 = p_sm8.tile([128, E], F32, tag="mask2", name="mask2")
            nc.vector.tensor_scalar(mask2[:], probs2[:], m2[:], None,
                                    op0=ALU.is_ge)
            st[g] = [xgb, m1, m2, mask1, mask2]

        def emit_tail_a1(g):
            xgb, m1, m2, mask1, mask2 = st[g]
            # positions: inclusive cumsum over tokens (partition dim) via
            # upper-triangular matmul, then mask to assigned experts
            pp = ps.tile([128, 2 * E], F32, tag="ps", name="ps")
            nc.tensor.matmul(pp[:, 0:E], utri[:], mask1[:],
                             start=True, stop=True)
            nc.tensor.matmul(pp[:, E:2 * E], utri[:], mask2[:],
                             start=True, stop=True)
            pos = []
            for ki, mask in enumerate((mask1, mask2)):
                pm = p_sm8.tile([128, E], F32, tag="pos", name="pos")
                nc.vector.tensor_mul(pm[:], pp[:, ki * E:(ki + 1) * E],
                                     mask[:])
                pos.append(pm)
            cmp1 = p_cmp.tile([128, E * CAP], BF16, tag="cmp1", name="cmp1")
            nc.vector.tensor_tensor(
                cmp1[:].rearrange("p (e c) -> p e c", e=E),
                pos[0][:].unsqueeze(2).broadcast_to([128, E, CAP]),
                iota_3d, op=ALU.is_equal)
            cmp2 = p_cmp.tile([128, E * CAP], BF16, tag="cmp2", name="cmp2")
            nc.vector.tensor_tensor(
                cmp2[:].rearrange("p (e c) -> p e c", e=E),
                pos[1][:].unsqueeze(2).broadcast_to([128, E, CAP]),
                iota_3d, op=ALU.is_equal)
            dm = p_dm.tile([128, E * CAP], BF16, tag="dm", name="dm")
            nc.vector.tensor_add(dm[:], cmp1[:], cmp2[:])
            st[g] += [cmp1, cmp2, dm]

        def emit_tail_a2(g):
            xgb, m1, m2, mask1, mask2, cmp1, cmp2, dm = st[g]
            # dispatch matmul for this group
            for dcp in range(4):
                dps = ps.tile([128, 512], F32, tag="ps", name="ps")
                for j in range(2):
                    dc = dcp * 2 + j
                    nc.tensor.matmul(dps[:, j * 256:(j + 1) * 256],
                                     xgb[:, dc * 128:(dc + 1) * 128],
                                     dm[:], start=True, stop=True)
                for j in range(2):
                    dc = dcp * 2 + j
                    dst = dispT[dc][:, g * E * CAP:(g + 1) * E * CAP]
                    if j == 0:
                        nc.vector.tensor_copy(dst, dps[:, 0:256])
                    else:
                        nc.scalar.copy(dst, dps[:, 256:512])
            cmp2s = p_cmp.tile([128, E * CAP], F32, tag="cmp2s", name="cmp2s")
            nc.vector.tensor_scalar_mul(cmp2s[:], cmp2[:], m2[:])
            comb = p_comb.tile([128, E * CAP], F32, tag="comb", name="comb")
            nc.vector.scalar_tensor_tensor(comb[:], cmp1[:], m1[:],
                                           cmp2s[:], ALU.mult, ALU.add)
            st[g] = [comb]

        def emit_tail_b(g):
            comb = st[g][-1]
            ctp = ps.tile([128, 256], F32, tag="ps", name="ps")
            for ch in range(2):
                nc.tensor.transpose(ctp[:, ch * 128:(ch + 1) * 128],
                                    comb[:, ch * 128:(ch + 1) * 128],
                                    ident[:])
            for ch in range(2):
                nc.vector.tensor_copy(combT[ch][:, g * 128:(g + 1) * 128],
                                      ctp[:, ch * 128:(ch + 1) * 128])
            del st[g]

        for g in range(4):
            emit_head(g)
        for q in range(4):
            for g in range(4 * q, 4 * q + 4):
                emit_tail_a1(g)
            for g in range(4 * q, 4 * q + 4):
                emit_tail_a2(g)
            if q < 3:
                for g in range(4 * (q + 1), 4 * (q + 1) + 4):
                    emit_head(g)
            _emit_ffn_unit(
                nc, pools,
                rhs_fn=lambda dc, q=q: xTb[dc][:, q * 512:(q + 1) * 512],
                out_ap_fn=lambda sc, q=q: sh_dram[q][sc * 128:
                                                    (sc + 1) * 128, :],
                keys_ap=sk_d, vals_ap=sv_d)
            for g in range(4 * q, 4 * q + 4):
                emit_tail_b(g)
        es_rt.close()

        # ---------- partial combine (experts 0-3 + shared), interleaved
        # between the later expert units; results land in the dead xTb
        # tiles so the final tail only reads experts 4-7 ----------
        p_eg0 = mk(es_ffn, "p_eg0", 8)
        p_shg0 = mk(es_ffn, "p_shg0", 8)
        part_tiles = {}

        def fetch_partial(g):
            eg0 = p_eg0.tile([128, D], BF16, tag="eg0", name="eg0")
            nc.sync.dma_start(
                eg0[:], eo_dram[0][g // 4][(g % 4) * 128:(g % 4 + 1) * 128, :])
            shg = p_shg0.tile([128, D], BF16, tag="shg0", name="shg0")
            nc.sync.dma_start(
                shg[:], sh_dram[g // 4][(g % 4) * 128:(g % 4 + 1) * 128, :])
            part_tiles[g] = (eg0, shg)

        def emit_partial(g):
            eg0, shg = part_tiles.pop(g)
            psA = ps.tile([128, 512], F32, tag="ps", name="ps")
            psB = ps.tile([128, 512], F32, tag="ps", name="ps")
            lhsT = combT[0][:, g * 128:(g + 1) * 128]
            nc.tensor.matmul(psA[:], lhsT, eg0[:, 0:512], start=True,
                             stop=True)
            nc.tensor.matmul(psB[:], lhsT, eg0[:, 512:1024], start=True,
                             stop=True)
            ra = xTb[g // 2][:, (g % 2) * 1024:(g % 2) * 1024 + 1024]
            nc.vector.tensor_tensor(ra[:, 0:512], psA[:], shg[:, 0:512],
                                    op=ALU.add)
            nc.vector.tensor_tensor(ra[:, 512:1024], psB[:],
                                    shg[:, 512:1024], op=ALU.add)

        # ---------- routed expert FFN units ----------
        for e in range(E):
            def rhs_fn(dc, e=e):
                r = dispT[dc][:].rearrange("p (g ec) -> p g ec", g=NG)
                return r[:, :, e * CAP:(e + 1) * CAP]
            def out_ap_fn(sc, e=e):
                # FFN2 tile rows are slots (g-major): row r -> group
                # 4*sc + r//32, capacity slot r%32 of expert e
                t = eo_dram[e // 4][sc]
                ap = t[:].rearrange("(gi ec) d -> gi ec d", ec=128)
                return ap[:, (e % 4) * CAP:(e % 4 + 1) * CAP, :]
            cb = None
            if e >= 4:
                def cb(e=e):
                    for g in range((e - 4) * 4, (e - 4) * 4 + 4):
                        fetch_partial(g)
            _emit_ffn_unit(
                nc, pools, rhs_fn=rhs_fn, out_ap_fn=out_ap_fn,
                keys_ap=k_d[e], vals_ap=v_d[e], pre_ffn2_cb=cb)
            if e >= 4:
                for g in range((e - 4) * 4, (e - 4) * 4 + 4):
                    emit_partial(g)
        es_ffn.close()
        es_dT.close()

        # ---------- final combine: experts 4-7 + the partial sums.
        # Two groups per DMA (paired reads/writes) to halve trigger count.
        p_eg = mk(es_cb, "p_eg", 6)
        p_ot = mk(es_cb, "p_ot", 3)
        for gp in range(NG // 2):
            g0 = gp * 2
            gi = g0 % 4
            egp = p_eg.tile([128, 2, D], BF16, tag="eg", name="eg")
            nc.sync.dma_start(
                egp[:],
                eo_dram[1][g0 // 4][gi * 128:(gi + 2) * 128, :]
                .rearrange("(i q) d -> q i d", i=2))
            ot = p_ot.tile([128, 2, D], BF16, tag="ot", name="ot")
            for i in range(2):
                g = g0 + i
                psA = ps.tile([128, 512], F32, tag="ps", name="ps")
                psB = ps.tile([128, 512], F32, tag="ps", name="ps")
                lhsT = combT[1][:, g * 128:(g + 1) * 128]
                nc.tensor.matmul(psA[:], lhsT, egp[:, i, 0:512],
                                 start=True, stop=True)
                nc.tensor.matmul(psB[:], lhsT, egp[:, i, 512:1024],
                                 start=True, stop=True)
                ra = xTb[g // 2][:, (g % 2) * 1024:(g % 2) * 1024 + 1024]
                nc.vector.tensor_tensor(ot[:, i, 0:512], psA[:],
                                        ra[:, 0:512], op=ALU.add)
                nc.vector.tensor_tensor(ot[:, i, 512:1024], psB[:],
                                        ra[:, 512:1024], op=ALU.add)
            nc.sync.dma_start(
                out_d[g0 * 128:(g0 + 2) * 128, :]
                .rearrange("(i q) d -> q i d", i=2), ot[:])
        es_cb.close()
        es_xtb.close()

    nc.compile()
    return nc


LAST_RESULT = None


def kernel(x, gate_weight, gate_bias, keys, values, shared_keys,
           shared_values, **_ignored):
    global LAST_RESULT
    _ensure_ntff_hook()
    x = np.ascontiguousarray(np.asarray(x, dtype=np.float32))
    gate_weight = np.ascontiguousarray(np.asarray(gate_weight, np.float32))
    gate_bias = np.ascontiguousarray(
        np.asarray(gate_bias, np.float32).reshape(1, E))
    k16 = np.ascontiguousarray(
        np.asarray(keys, np.float32).astype(ml_dtypes.bfloat16))
    v16 = np.ascontiguousarray(
        np.asarray(values, np.float32).astype(ml_dtypes.bfloat16))
    shk16 = np.ascontiguousarray(
        np.asarray(shared_keys, np.float32).reshape(D, H)
        .astype(ml_dtypes.bfloat16))
    shv16 = np.ascontiguousarray(
        np.asarray(shared_values, np.float32).reshape(H, D)
        .astype(ml_dtypes.bfloat16))

    if "nc" not in _CACHE:
        _CACHE["nc"] = _build_program()
    nc = _CACHE["nc"]

    xt = x.reshape(NCORES, TOK, D)
    in_maps = []
    for i in range(NCORES):
        in_maps.append({
            "x_s": np.ascontiguousarray(xt[i]),
            "gw": gate_weight,
            "gb": gate_bias,
            "k16": k16,
            "v16": v16,
            "shk16": shk16,
            "shv16": shv16,
        })
    res = run_bass_kernel_spmd(nc, in_maps, core_ids=list(range(NCORES)))
    LAST_RESULT = res
    out = np.concatenate([res.results[i]["out"] for i in range(NCORES)],
                         axis=0)
    return out.reshape(B, S, D).astype(np.float32)
